# revision 1
# baseline (speedup 1.0000x reference)
"""GroupedExpertNetwork (SwiGLU per-expert MLP) Trainium2 kernel.

Expert-parallel: 8 experts -> 8 NeuronCores, one expert per core.
Per core:  g = x @ gate; u = x @ inner; h = silu(g)*u; out = h @ outp
Shapes per expert: x [T=2048, D=2048], gate/inner [D, I=4096], outp [I, D].

Strategy:
- Host: transpose x -> xT [D, T], cast everything to bf16, and pre-pack all
  streamed tensors into tile-contiguous layouts so each device DMA is one
  contiguous block.
- Device: T-blocked (TB=512). For each T block, compute hT [I, TB] on-chip
  (kept fully resident in SBUF, bf16), then the output matmuls accumulate
  over the full I=4096 in PSUM. All matmul free dims are >=256 so bf16 runs
  at 1 cycle/row on the PE. PSUM accumulates in f32; output is f32.
"""

import numpy as np
import ml_dtypes

E, T, D, I = 8, 2048, 2048, 4096
P = 128
TB = 512                 # T block size
NT = T // TB             # 4 T blocks
IC = 256                 # I chunk for gate/inner weight streaming
NIC = I // IC            # 16
DC = 512                 # D chunk for output weight streaming
NDC = D // DC            # 4
KD = D // P              # 16 contraction chunks for gate/inner matmuls
KI = I // P              # 32 contraction chunks for output matmul

_COMPILED = None


def _build_program():
    import concourse.mybir as mybir
    import concourse.tile as tile
    from concourse import bacc

    bf16 = mybir.dt.bfloat16
    f32 = mybir.dt.float32

    nc = bacc.Bacc(
        "TRN2",
        target_bir_lowering=False,
        debug=False,
        num_devices=E,
    )

    # Packed DRAM inputs (per core = one expert):
    # xt:  [NT, P, KD, TB]   xT tiles, d = ko*128+p
    # gw:  [NIC, P, KD, IC]  gate tiles
    # uw:  [NIC, P, KD, IC]  inner tiles
    # ow:  [NDC, P, KI, DC]  output-proj tiles
    xt_d = nc.dram_tensor("xt", (NT, P, KD, TB), bf16, kind="ExternalInput")
    gw_d = nc.dram_tensor("gw", (NIC, P, KD, IC), bf16, kind="ExternalInput")
    uw_d = nc.dram_tensor("uw", (NIC, P, KD, IC), bf16, kind="ExternalInput")
    ow_d = nc.dram_tensor("ow", (NDC, P, KI, DC), bf16, kind="ExternalInput")
    out_d = nc.dram_tensor("out", (T, D), f32, kind="ExternalOutput")

    xt_ap = xt_d.ap()
    gw_ap = gw_d.ap()
    uw_ap = uw_d.ap()
    ow_ap = ow_d.ap()
    # out rows = tb*128 + p
    out_ap = out_d.ap().rearrange("(tb p) d -> tb p d", p=P)

    MI = IC // P  # 2 mi groups per I chunk

    with tile.TileContext(nc) as tc:
        with (
            tc.tile_pool(name="xt", bufs=2) as xt_pool,
            tc.tile_pool(name="w", bufs=2) as w_pool,
            tc.tile_pool(name="ow", bufs=2) as ow_pool,
            tc.tile_pool(name="ht", bufs=1) as ht_pool,
            tc.tile_pool(name="tmp", bufs=3) as tmp_pool,
            tc.tile_pool(name="osb", bufs=3) as osb_pool,
            tc.tile_pool(name="pg", bufs=2, space="PSUM") as pg_pool,
            tc.tile_pool(name="pu", bufs=2, space="PSUM") as pu_pool,
            tc.tile_pool(name="po", bufs=2, space="PSUM") as po_pool,
        ):
            for tblk in range(NT):
                xt = xt_pool.tile([P, KD, TB], bf16, tag="xt")
                nc.sync.dma_start(xt[:], xt_ap[tblk])

                ht = ht_pool.tile([P, KI, TB], bf16, tag="ht")

                for ic in range(NIC):
                    gw = w_pool.tile([P, KD, IC], bf16, tag="gw")
                    nc.sync.dma_start(gw[:], gw_ap[ic])
                    uw = w_pool.tile([P, KD, IC], bf16, tag="uw")
                    nc.sync.dma_start(uw[:], uw_ap[ic])

                    for mi in range(MI):
                        pg = pg_pool.tile([P, TB], f32, tag="pg")
                        for k in range(KD):
                            nc.tensor.matmul(
                                pg[:],
                                gw[:, k, mi * P:(mi + 1) * P],
                                xt[:, k, :],
                                start=(k == 0),
                                stop=(k == KD - 1),
                            )
                        pu = pu_pool.tile([P, TB], f32, tag="pu")
                        for k in range(KD):
                            nc.tensor.matmul(
                                pu[:],
                                uw[:, k, mi * P:(mi + 1) * P],
                                xt[:, k, :],
                                start=(k == 0),
                                stop=(k == KD - 1),
                            )
                        tmp = tmp_pool.tile([P, TB], f32, tag="tmp")
                        nc.scalar.activation(
                            tmp[:], pg[:], mybir.ActivationFunctionType.Silu
                        )
                        nc.vector.tensor_tensor(
                            ht[:, ic * MI + mi, :],
                            tmp[:],
                            pu[:],
                            mybir.AluOpType.mult,
                        )

                for dc in range(NDC):
                    ow = ow_pool.tile([P, KI, DC], bf16, tag="ow")
                    nc.sync.dma_start(ow[:], ow_ap[dc])
                    for ti in range(TB // P):
                        po = po_pool.tile([P, DC], f32, tag="po")
                        for k in range(KI):
                            nc.tensor.matmul(
                                po[:],
                                ht[:, k, ti * P:(ti + 1) * P],
                                ow[:, k, :],
                                start=(k == 0),
                                stop=(k == KI - 1),
                            )
                        osb = osb_pool.tile([P, DC], f32, tag="osb")
                        nc.vector.tensor_copy(osb[:], po[:])
                        nc.sync.dma_start(
                            out_ap[tblk * (TB // P) + ti, :, dc * DC:(dc + 1) * DC],
                            osb[:],
                        )

    nc.compile()
    return nc


def _get_program():
    global _COMPILED
    if _COMPILED is None:
        _COMPILED = _build_program()
    return _COMPILED


def _pack_inputs(x, gate_proj, inner_proj, output_proj):
    bf16 = ml_dtypes.bfloat16
    in_maps = []
    for e in range(E):
        # xT [D, T] -> [NT, P, KD, TB]; d = ko*P + p
        xt = np.ascontiguousarray(x[e].T).astype(bf16)
        xt = xt.reshape(KD, P, NT, TB).transpose(2, 1, 0, 3)
        xt = np.ascontiguousarray(xt)
        # gate [D, I] -> [NIC, P, KD, IC]
        gw = gate_proj[e].astype(bf16).reshape(KD, P, NIC, IC).transpose(2, 1, 0, 3)
        gw = np.ascontiguousarray(gw)
        uw = inner_proj[e].astype(bf16).reshape(KD, P, NIC, IC).transpose(2, 1, 0, 3)
        uw = np.ascontiguousarray(uw)
        # outp [I, D] -> [NDC, P, KI, DC]
        ow = output_proj[e].astype(bf16).reshape(KI, P, NDC, DC).transpose(2, 1, 0, 3)
        ow = np.ascontiguousarray(ow)
        in_maps.append({"xt": xt, "gw": gw, "uw": uw, "ow": ow})
    return in_maps


def kernel(x, gate_proj, inner_proj, output_proj, _trace=False, _trace_kwargs=None):
    from concourse import bass_utils

    nc = _get_program()
    in_maps = _pack_inputs(
        np.asarray(x), np.asarray(gate_proj), np.asarray(inner_proj),
        np.asarray(output_proj),
    )
    res = bass_utils.run_bass_kernel_spmd(
        nc,
        in_maps,
        core_ids=list(range(E)),
        trace=_trace,
        **(_trace_kwargs or {}),
    )
    out = np.stack([np.asarray(res.results[e]["out"]) for e in range(E)])
    if _trace:
        return out.astype(np.float32, copy=False), res
    return out.astype(np.float32, copy=False)


# revision 3
# speedup vs baseline: 1.3794x; 1.3794x over previous
"""GroupedExpertNetwork (SwiGLU per-expert MLP) Trainium2 kernel.

Expert-parallel: 8 experts -> 8 NeuronCores, one expert per core.
Per core:  g = x @ gate; u = x @ inner; h = silu(g)*u; out = h @ outp
Shapes per expert: x [T=2048, D=2048], gate/inner [D, I=4096], outp [I, D].

Strategy:
- Host: transpose x -> xT [D, T], cast everything to bf16, and pre-pack all
  streamed tensors into tile-contiguous layouts so each device DMA is one
  contiguous block.
- Device: T-blocked (TB=512). For each T block, compute hT [I, TB] on-chip
  (kept fully resident in SBUF, bf16), then the output matmuls accumulate
  over the full I=4096 in PSUM. All matmul free dims are >=256 so bf16 runs
  at 1 cycle/row on the PE. PSUM accumulates in f32; output is f32.
"""

import numpy as np
import ml_dtypes

E, T, D, I = 8, 2048, 2048, 4096
P = 128
TB = 512                 # T block size
NT = T // TB             # 4 T blocks
IC = 256                 # I chunk for gate/inner weight streaming
NIC = I // IC            # 16
DC = 512                 # D chunk for output weight streaming
NDC = D // DC            # 4
KD = D // P              # 16 contraction chunks for gate/inner matmuls
KI = I // P              # 32 contraction chunks for output matmul

_COMPILED = None


def _build_program(reps=1):
    import concourse.mybir as mybir
    import concourse.tile as tile
    from concourse import bacc

    bf16 = mybir.dt.bfloat16
    f32 = mybir.dt.float32

    nc = bacc.Bacc(
        "TRN2",
        target_bir_lowering=False,
        debug=False,
        num_devices=E,
    )

    # Packed DRAM inputs (per core = one expert):
    # xt:  [NT, P, KD, TB]   xT tiles, d = ko*128+p
    # gw:  [NIC, P, KD, IC]  gate tiles
    # uw:  [NIC, P, KD, IC]  inner tiles
    # ow:  [NDC, P, KI, DC]  output-proj tiles
    xt_d = nc.dram_tensor("xt", (NT, P, KD, TB), bf16, kind="ExternalInput")
    gw_d = nc.dram_tensor("gw", (NIC, P, KD, IC), bf16, kind="ExternalInput")
    uw_d = nc.dram_tensor("uw", (NIC, P, KD, IC), bf16, kind="ExternalInput")
    ow_d = nc.dram_tensor("ow", (NDC, P, KI, DC), bf16, kind="ExternalInput")
    out_d = nc.dram_tensor("out", (T, D), f32, kind="ExternalOutput")

    xt_ap = xt_d.ap()
    gw_ap = gw_d.ap()
    uw_ap = uw_d.ap()
    ow_ap = ow_d.ap()
    # out rows = tb*128 + p
    out_ap = out_d.ap().rearrange("(tb p) d -> tb p d", p=P)

    MI = IC // P  # 2 mi groups per I chunk

    with tile.TileContext(nc) as tc:
        with (
            tc.tile_pool(name="xt", bufs=2) as xt_pool,
            tc.tile_pool(name="w", bufs=2) as w_pool,
            tc.tile_pool(name="ow", bufs=2) as ow_pool,
            tc.tile_pool(name="ht", bufs=1) as ht_pool,
            tc.tile_pool(name="tmp", bufs=3) as tmp_pool,
            tc.tile_pool(name="osb", bufs=3) as osb_pool,
            tc.tile_pool(name="pg", bufs=2, space="PSUM") as pg_pool,
            tc.tile_pool(name="pu", bufs=2, space="PSUM") as pu_pool,
            tc.tile_pool(name="po", bufs=2, space="PSUM") as po_pool,
        ):
          for _rep in range(reps):
            for tblk in range(NT):
                xt = xt_pool.tile([P, KD, TB], bf16, tag="xt")
                nc.sync.dma_start(xt[:], xt_ap[tblk])

                ht = ht_pool.tile([P, KI, TB], bf16, tag="ht")

                for ic in range(NIC):
                    gw = w_pool.tile([P, KD, IC], bf16, tag="gw")
                    nc.sync.dma_start(gw[:], gw_ap[ic])
                    uw = w_pool.tile([P, KD, IC], bf16, tag="uw")
                    nc.sync.dma_start(uw[:], uw_ap[ic])

                    for mi in range(MI):
                        pg = pg_pool.tile([P, TB], f32, tag="pg")
                        for k in range(KD):
                            nc.tensor.matmul(
                                pg[:],
                                gw[:, k, mi * P:(mi + 1) * P],
                                xt[:, k, :],
                                start=(k == 0),
                                stop=(k == KD - 1),
                            )
                        pu = pu_pool.tile([P, TB], f32, tag="pu")
                        for k in range(KD):
                            nc.tensor.matmul(
                                pu[:],
                                uw[:, k, mi * P:(mi + 1) * P],
                                xt[:, k, :],
                                start=(k == 0),
                                stop=(k == KD - 1),
                            )
                        tmp = tmp_pool.tile([P, TB], f32, tag="tmp")
                        nc.scalar.activation(
                            tmp[:], pg[:], mybir.ActivationFunctionType.Silu
                        )
                        nc.vector.tensor_tensor(
                            ht[:, ic * MI + mi, :],
                            tmp[:],
                            pu[:],
                            mybir.AluOpType.mult,
                        )

                for dc in range(NDC):
                    ow = ow_pool.tile([P, KI, DC], bf16, tag="ow")
                    nc.sync.dma_start(ow[:], ow_ap[dc])
                    for ti in range(TB // P):
                        po = po_pool.tile([P, DC], f32, tag="po")
                        for k in range(KI):
                            nc.tensor.matmul(
                                po[:],
                                ht[:, k, ti * P:(ti + 1) * P],
                                ow[:, k, :],
                                start=(k == 0),
                                stop=(k == KI - 1),
                            )
                        osb = osb_pool.tile([P, DC], f32, tag="osb")
                        nc.vector.tensor_copy(osb[:], po[:])
                        nc.sync.dma_start(
                            out_ap[tblk * (TB // P) + ti, :, dc * DC:(dc + 1) * DC],
                            osb[:],
                        )

    nc.compile()
    return nc


def _get_program():
    global _COMPILED
    if _COMPILED is None:
        _COMPILED = _build_program()
    return _COMPILED


def _pack_inputs(x, gate_proj, inner_proj, output_proj):
    bf16 = ml_dtypes.bfloat16
    in_maps = []
    for e in range(E):
        # xT [D, T] -> [NT, P, KD, TB]; d = ko*P + p
        xt = np.ascontiguousarray(x[e].T).astype(bf16)
        xt = xt.reshape(KD, P, NT, TB).transpose(2, 1, 0, 3)
        xt = np.ascontiguousarray(xt)
        # gate [D, I] -> [NIC, P, KD, IC]
        gw = gate_proj[e].astype(bf16).reshape(KD, P, NIC, IC).transpose(2, 1, 0, 3)
        gw = np.ascontiguousarray(gw)
        uw = inner_proj[e].astype(bf16).reshape(KD, P, NIC, IC).transpose(2, 1, 0, 3)
        uw = np.ascontiguousarray(uw)
        # outp [I, D] -> [NDC, P, KI, DC]
        ow = output_proj[e].astype(bf16).reshape(KI, P, NDC, DC).transpose(2, 1, 0, 3)
        ow = np.ascontiguousarray(ow)
        in_maps.append({"xt": xt, "gw": gw, "uw": uw, "ow": ow})
    return in_maps


def kernel(x, gate_proj, inner_proj, output_proj, _trace=False, _trace_kwargs=None):
    from concourse import bass_utils

    nc = _get_program()
    in_maps = _pack_inputs(
        np.asarray(x), np.asarray(gate_proj), np.asarray(inner_proj),
        np.asarray(output_proj),
    )
    res = bass_utils.run_bass_kernel_spmd(
        nc,
        in_maps,
        core_ids=list(range(E)),
        trace=_trace,
        **(_trace_kwargs or {}),
    )
    out = np.stack([np.asarray(res.results[e]["out"]) for e in range(E)])
    if _trace:
        return out.astype(np.float32, copy=False), res
    return out.astype(np.float32, copy=False)


# revision 6
# speedup vs baseline: 2.3100x; 1.6747x over previous
"""GroupedExpertNetwork (SwiGLU per-expert MLP) Trainium2 kernel.

Expert-parallel: 8 experts -> 8 NeuronCores, one expert per core.
Per core:  g = x @ gate; u = x @ inner; h = silu(g)*u; out = h @ outp
Shapes per expert: x [T=2048, D=2048], gate/inner [D, I=4096], outp [I, D].

Strategy:
- Host: transpose x -> xT [D, T], cast everything to bf16, and pre-pack all
  streamed tensors into tile-contiguous layouts so each device DMA is one
  contiguous block.
- Device: T-blocked (TB=512). For each T block, compute hT [I, TB] on-chip
  (kept fully resident in SBUF, bf16), then the output matmuls accumulate
  over the full I=4096 in PSUM. All matmul free dims are >=256 so bf16 runs
  at 1 cycle/row on the PE. PSUM accumulates in f32; output is f32.
"""

import numpy as np
import ml_dtypes

E, T, D, I = 8, 2048, 2048, 4096
P = 128
TB = 512                 # T block size
NT = T // TB             # 4 T blocks
IC = 256                 # I chunk for gate/inner weight streaming
NIC = I // IC            # 16
DC = 512                 # D chunk for output weight streaming
NDC = D // DC            # 4
KD = D // P              # 16 contraction chunks for gate/inner matmuls
KI = I // P              # 32 contraction chunks for output matmul

_COMPILED = None


def _build_program(reps=1, skip_mm=False, skip_dma=False):
    import concourse.mybir as mybir
    import concourse.tile as tile
    from concourse import bacc

    bf16 = mybir.dt.bfloat16
    f32 = mybir.dt.float32

    nc = bacc.Bacc(
        "TRN2",
        target_bir_lowering=False,
        debug=False,
        num_devices=E,
    )

    # Packed DRAM inputs (per core = one expert):
    # xt:  [NT, P, KD, TB]   xT tiles, d = ko*128+p
    # gw:  [NIC, P, KD, IC]  gate tiles
    # uw:  [NIC, P, KD, IC]  inner tiles
    # ow:  [NDC, P, KI, DC]  output-proj tiles
    xt_d = nc.dram_tensor("xt", (NT, P, KD, TB), bf16, kind="ExternalInput")
    gw_d = nc.dram_tensor("gw", (NIC, P, KD, IC), bf16, kind="ExternalInput")
    uw_d = nc.dram_tensor("uw", (NIC, P, KD, IC), bf16, kind="ExternalInput")
    ow_d = nc.dram_tensor("ow", (NDC, P, KI, DC), bf16, kind="ExternalInput")
    out_d = nc.dram_tensor("out", (T, D), f32, kind="ExternalOutput")

    xt_ap = xt_d.ap()
    gw_ap = gw_d.ap()
    uw_ap = uw_d.ap()
    ow_ap = ow_d.ap()
    # out rows = tb*128 + p
    out_ap = out_d.ap().rearrange("(tb p) d -> tb p d", p=P)

    MI = IC // P  # 2 mi groups per I chunk

    with tile.TileContext(nc) as tc:
        with (
            tc.tile_pool(name="xt", bufs=2) as xt_pool,
            tc.tile_pool(name="w", bufs=2) as w_pool,
            tc.tile_pool(name="ow", bufs=2) as ow_pool,
            tc.tile_pool(name="ht", bufs=1) as ht_pool,
            tc.tile_pool(name="tmp", bufs=3) as tmp_pool,
            tc.tile_pool(name="osb", bufs=3) as osb_pool,
            tc.tile_pool(name="pg", bufs=2, space="PSUM") as pg_pool,
            tc.tile_pool(name="pu", bufs=2, space="PSUM") as pu_pool,
            tc.tile_pool(name="po", bufs=2, space="PSUM") as po_pool,
        ):
          for _rep in range(reps):
            for tblk in range(NT):
                xt = xt_pool.tile([P, KD, TB], bf16, tag="xt")
                if not skip_dma:
                    nc.sync.dma_start(xt[:], xt_ap[tblk])

                ht = ht_pool.tile([P, KI, TB], bf16, tag="ht")

                for ic in range(NIC):
                    gw = w_pool.tile([P, KD, IC], bf16, tag="gw")
                    uw = w_pool.tile([P, KD, IC], bf16, tag="uw")
                    if not skip_dma:
                        nc.sync.dma_start(gw[:], gw_ap[ic])
                        nc.sync.dma_start(uw[:], uw_ap[ic])

                    for mi in range(MI):
                        pg = pg_pool.tile([P, TB], f32, tag="pg")
                        if not skip_mm:
                            for k in range(KD):
                                nc.tensor.matmul(
                                    pg[:],
                                    gw[:, k, mi * P:(mi + 1) * P],
                                    xt[:, k, :],
                                    start=(k == 0),
                                    stop=(k == KD - 1),
                                )
                        pu = pu_pool.tile([P, TB], f32, tag="pu")
                        if not skip_mm:
                            for k in range(KD):
                                nc.tensor.matmul(
                                    pu[:],
                                    uw[:, k, mi * P:(mi + 1) * P],
                                    xt[:, k, :],
                                    start=(k == 0),
                                    stop=(k == KD - 1),
                                )
                        if not skip_mm:
                            tmp = tmp_pool.tile([P, TB], f32, tag="tmp")
                            nc.scalar.activation(
                                tmp[:], pg[:], mybir.ActivationFunctionType.Silu
                            )
                            nc.vector.tensor_tensor(
                                ht[:, ic * MI + mi, :],
                                tmp[:],
                                pu[:],
                                mybir.AluOpType.mult,
                            )

                for dc in range(NDC):
                    ow = ow_pool.tile([P, KI, DC], bf16, tag="ow")
                    if not skip_dma:
                        nc.sync.dma_start(ow[:], ow_ap[dc])
                    for ti in range(TB // P):
                        po = po_pool.tile([P, DC], f32, tag="po")
                        if not skip_mm:
                            for k in range(KI):
                                nc.tensor.matmul(
                                    po[:],
                                    ht[:, k, ti * P:(ti + 1) * P],
                                    ow[:, k, :],
                                    start=(k == 0),
                                    stop=(k == KI - 1),
                                )
                        if not skip_mm:
                            osb = osb_pool.tile([P, DC], f32, tag="osb")
                            nc.vector.tensor_copy(osb[:], po[:])
                            nc.sync.dma_start(
                                out_ap[tblk * (TB // P) + ti, :, dc * DC:(dc + 1) * DC],
                                osb[:],
                            )

    nc.compile()
    return nc


def _get_program():
    global _COMPILED
    if _COMPILED is None:
        _COMPILED = _build_program()
    return _COMPILED


def _pack_inputs(x, gate_proj, inner_proj, output_proj):
    bf16 = ml_dtypes.bfloat16
    in_maps = []
    for e in range(E):
        # xT [D, T] -> [NT, P, KD, TB]; d = ko*P + p
        xt = np.ascontiguousarray(x[e].T).astype(bf16)
        xt = xt.reshape(KD, P, NT, TB).transpose(2, 1, 0, 3)
        xt = np.ascontiguousarray(xt)
        # gate [D, I] -> [NIC, P, KD, IC]
        gw = gate_proj[e].astype(bf16).reshape(KD, P, NIC, IC).transpose(2, 1, 0, 3)
        gw = np.ascontiguousarray(gw)
        uw = inner_proj[e].astype(bf16).reshape(KD, P, NIC, IC).transpose(2, 1, 0, 3)
        uw = np.ascontiguousarray(uw)
        # outp [I, D] -> [NDC, P, KI, DC]
        ow = output_proj[e].astype(bf16).reshape(KI, P, NDC, DC).transpose(2, 1, 0, 3)
        ow = np.ascontiguousarray(ow)
        in_maps.append({"xt": xt, "gw": gw, "uw": uw, "ow": ow})
    return in_maps


def kernel(x, gate_proj, inner_proj, output_proj, _trace=False, _trace_kwargs=None):
    from concourse import bass_utils

    nc = _get_program()
    in_maps = _pack_inputs(
        np.asarray(x), np.asarray(gate_proj), np.asarray(inner_proj),
        np.asarray(output_proj),
    )
    res = bass_utils.run_bass_kernel_spmd(
        nc,
        in_maps,
        core_ids=list(range(E)),
        trace=_trace,
        **(_trace_kwargs or {}),
    )
    out = np.stack([np.asarray(res.results[e]["out"]) for e in range(E)])
    if _trace:
        return out.astype(np.float32, copy=False), res
    return out.astype(np.float32, copy=False)
